# revision 38
# baseline (speedup 1.0000x reference)
"""DepthAwareConv2d Trainium2 kernel.

Math: the reference's depth-modulated im2col GEMM is exactly
    out = conv2d(x * depth, weight, stride=1, pad=1) + bias
(depth broadcasts over channels; unfold(x)*unfold(depth) = unfold(x*depth)).

Sharding (8 cores): data-parallel over N (4 images) x spatial-parallel over
image row halves.  Core cid handles n = cid//2, row half = cid%2 (output rows
[0,64) or [64,128)), computing all 256 output channels for its half.  The
host ships each core its 64 input rows plus one halo/zero row on each side
(66 rows total), so the device program is identical on every core (SPMD) and
no collectives are needed.

Per-core device kernel:
  1. DMA the 66 x-rows and the matching partition-broadcast depth rows into
     SBUF chunks; DVE-multiply them into a column-padded fp32r image
     ypad (C=128 partitions, 66 x 130).
  2. Shift-conv: per 4-row output block and 128-wide out-channel block,
     9 accumulating fp32r matmuls (stationary = 128x128 weight tap, moving =
     shifted 4x128 window, free dim 512 = full-rate fp32 on the PE) into one
     PSUM bank.
  3. ScalarE Identity(+bias) PSUM->SBUF, DMA out.
"""

import numpy as np

import concourse.bass as bass
import concourse.mybir as mybir
import concourse.tile as tile
from concourse import bacc
from concourse.bass_utils import run_bass_kernel_spmd

N, C, O, H, W = 4, 128, 256, 128, 128
HSH = H // 2  # output rows per core
HIN = HSH + 2  # input rows per core incl. halo/zero rows
NCORES = 8
F32 = mybir.dt.float32
F32R = mybir.dt.float32r
ACT_IDENT = mybir.ActivationFunctionType.Identity

RB = 4  # output rows per matmul tile (free dim RB*W = 512, one PSUM bank)
# image rows per load/multiply chunk; a small first chunk lets the first
# matmul block (which only needs rows 0..5) start as early as possible
CHUNKS = (8, 8, 12, 12, 12, 14)  # sums to HIN = 66; boundaries all 0 mod 4

_CACHE = {}


def build_nc():
    nc = bacc.Bacc("TRN2", target_bir_lowering=False, debug=False, num_devices=NCORES)
    xs = nc.declare_dram_parameter("xs", [C, HIN, W], F32, isOutput=False)
    dep = nc.declare_dram_parameter("dep", [HIN * W], F32, isOutput=False)
    wt = nc.declare_dram_parameter("wt", [C, 2, 9, O // 2], F32, isOutput=False)
    bb = nc.declare_dram_parameter("bb", [O // 2, 2], F32, isOutput=False)
    out = nc.declare_dram_parameter("out", [O, HSH, W], F32, isOutput=True)

    with tile.TileContext(nc) as tc:
        with (
            tc.tile_pool(name="big", bufs=1) as big,
            tc.tile_pool(name="wp", bufs=1) as wp,
            tc.tile_pool(name="ch", bufs=3) as chp,
            tc.tile_pool(name="op", bufs=4) as op,
            tc.tile_pool(name="pp", bufs=8, space="PSUM") as pp,
        ):
            # fp32r tiles: every writer must emit fp32r-rounded values
            # (BIR verifier rule for FP32r matmult inputs), so x*depth and
            # the weights are written by compute ops with fp32r out dtype.
            ypad = big.tile([C, HIN, W + 2], F32R)
            wsb = wp.tile([C, 2, 9, O // 2], F32R)
            wtmp = wp.tile([C, 2, 9, O // 2], F32)
            bsb = wp.tile([O // 2, 2], F32)  # bsb[p, ob] = bias[ob*128 + p]

            # zeros staging: borders of ypad and the PE warm-up operand.
            # Memset can't write fp32r, so round-copy zeros via DVE.
            ztile = wp.tile([C, RB * W], F32)
            zr = wp.tile([C, RB * W], F32R)
            nc.vector.memset(ztile, 0.0)
            nc.vector.tensor_copy(out=zr, in_=ztile)
            nc.vector.tensor_copy(out=ypad[:, :, 0], in_=ztile[:, :HIN])
            nc.vector.tensor_copy(out=ypad[:, :, W + 1], in_=ztile[:, :HIN])

            # PE warm-up: ~4us of zero matmuls while the input DMA streams in,
            # so the HAM clock gate is already at full rate (2.4 GHz) when the
            # real matmuls start.
            # depth rows 0..15 for the PE-side broadcast of chunks 0/1 (a K=1
            # ones-matmul replicates one depth row across all 128 partitions
            # into PSUM, so the startup chunks don't need the 0.5MB/chunk
            # DMA broadcast in the critical head window)
            NBC = 2  # chunks 0..NBC-1 use the PE broadcast
            bcrows = sum(CHUNKS[:NBC])
            dsm = wp.tile([1, bcrows * W], F32)
            ones1 = wp.tile([1, 128], F32)
            nc.sync.dma_start(out=dsm, in_=dep.ap()[None, : bcrows * W])
            nc.vector.memset(ones1, 1.0)
            dbc_ps = []
            for k in range(bcrows * W // 512):
                psb = pp.tile([C, RB, W], F32, tag="ps", name=f"psb{k}")
                nc.tensor.matmul(
                    psb, ones1, dsm[:, k * 512 : (k + 1) * 512], start=True, stop=True
                )
                dbc_ps.append(psb)

            warm = pp.tile([O // 2, RB, W], F32, tag="ps")
            for _ in range(4):
                nc.tensor.matmul(warm, zr[:, :128], zr, start=True, stop=True)

            CMAX = max(CHUNKS)
            bases = []
            b = 0
            for ch in CHUNKS:
                bases.append(b)
                b += ch
            tiles = {}

            def chunk_dma(ci):
                r0, ch = bases[ci], CHUNKS[ci]
                xb = chp.tile([C, CMAX, W], F32, tag="xb", name=f"xb{ci}")
                nc.sync.dma_start(out=xb[:, :ch], in_=xs[:, r0 : r0 + ch, :])
                if ci < NBC:
                    tiles[ci] = (xb, None)
                    return
                db = chp.tile([C, CMAX, W], F32, tag="db", name=f"db{ci}")
                nc.sync.dma_start(
                    out=db[:, :ch],
                    in_=dep.ap()[r0 * W : (r0 + ch) * W].partition_broadcast(C),
                )
                tiles[ci] = (xb, db)

            def mul_rows(r0, r1):
                # 4-aligned multiply blocks: conv block t reads ypad rows
                # 4t..4t+5; keeping writer granularity 4-aligned means Tile's
                # (quantized) range-overlap check never drags in a writer one
                # byte past the true read range.
                ci = next(
                    i for i, base in enumerate(bases) if base <= r0 < base + CHUNKS[i]
                )
                xb, db = tiles[ci]
                lo = r0 - bases[ci]
                if db is None:
                    d_ap = dbc_ps[r0 // 4][:, : r1 - r0]
                else:
                    d_ap = db[:, lo : lo + (r1 - r0)]
                nc.vector.tensor_mul(
                    out=ypad[:, r0:r1, 1 : W + 1],
                    in0=xb[:, lo : lo + (r1 - r0)],
                    in1=d_ap,
                )

            # Startup-critical transfers (weights half 0 + chunk 0) are split
            # into ~0.26MB pieces dual-dispatched on the two HWDGE engines
            # (sync + scalar): a single dma_start only sustains ~180GB/s and
            # each dispatch costs ~0.6us of sequencer time, so saturating the
            # ~435GB/s of SBUF ports needs several concurrent dma_starts.
            xb0 = chp.tile([C, CMAX, W], F32, tag="xb", name="xb0")
            tiles[0] = (xb0, None)
            ch0 = CHUNKS[0]
            nc.sync.dma_start(out=xb0[:, : ch0 // 2], in_=xs[:, : ch0 // 2, :])
            nc.scalar.dma_start(out=wtmp[:, 0, :5], in_=wt.ap()[:, 0, :5])
            nc.sync.dma_start(out=xb0[:, ch0 // 2 : ch0], in_=xs[:, ch0 // 2 : ch0, :])
            nc.scalar.copy(out=wsb[:, 0, :5], in_=wtmp[:, 0, :5])
            mul_rows(0, 4)
            mul_rows(4, 8)

            # second wave: weight tail, chunk 1, bias
            nc.scalar.dma_start(out=wtmp[:, 0, 5:], in_=wt.ap()[:, 0, 5:])
            nc.scalar.copy(out=wsb[:, 0, 5:], in_=wtmp[:, 0, 5:])
            nc.scalar.dma_start(out=wtmp[:, 1, :5], in_=wt.ap()[:, 1, :5])
            nc.scalar.dma_start(out=wtmp[:, 1, 5:], in_=wt.ap()[:, 1, 5:])
            chunk_dma(1)
            nc.scalar.copy(out=wsb[:, 1], in_=wtmp[:, 1])
            nc.scalar.dma_start(out=bsb, in_=bb.ap())

            for ci in range(2, len(CHUNKS)):
                with tc.tile_wait_until(0.004 * (ci - 1)):
                    chunk_dma(ci)
            r = CHUNKS[0]
            while r < HIN:
                r1 = min(r + 4, HIN)
                mul_rows(r, r1)
                r = r1

            for rb in range(0, HSH, RB):
                osb = op.tile([O // 2, 2, RB, W], F32)
                for ob in range(2):
                    ps = pp.tile([O // 2, RB, W], F32, tag="ps", name=f"ps{rb}_{ob}")
                    for p in range(9):
                        i, j = divmod(p, 3)
                        nc.tensor.matmul(
                            ps,
                            wsb[:, ob, p],
                            ypad[:, rb + i : rb + i + RB, j : j + W],
                            start=(p == 0),
                            stop=(p == 8),
                        )
                    nc.scalar.activation(
                        out=osb[:, ob],
                        in_=ps,
                        func=ACT_IDENT,
                        bias=bsb[:, ob : ob + 1],
                        scale=1.0,
                    )
                # osb holds (128 partitions, [ob, row, col]); DRAM wants
                # (o, row, col) with o = ob*128 + partition.
                nc.sync.dma_start(
                    out=out[:, rb : rb + RB, :].rearrange(
                        "(ob o) r w -> o ob r w", ob=2
                    ),
                    in_=osb,
                )

    nc.compile()
    return nc


def _get_nc():
    if "nc" not in _CACHE:
        _CACHE["nc"] = build_nc()
    return _CACHE["nc"]


def make_in_maps(x, depth, weight, bias):
    x = np.asarray(x, np.float32)
    depth = np.asarray(depth, np.float32)
    weight = np.asarray(weight, np.float32)
    bias = np.asarray(bias, np.float32)
    # (O, C, 3, 3) -> (C, ob, tap=i*3+j, o) with o = local index in the
    # 128-wide out-channel half ob
    wt9 = np.ascontiguousarray(
        np.transpose(
            weight.reshape(2, O // 2, C, 3, 3), (2, 0, 3, 4, 1)
        ).reshape(C, 2, 9, O // 2)
    )
    bb = np.ascontiguousarray(bias.reshape(2, O // 2).T)
    in_maps = []
    for cid in range(NCORES):
        n, hh = divmod(cid, 2)
        xsh = np.zeros((C, HIN, W), np.float32)
        dsh = np.zeros((HIN, W), np.float32)
        if hh == 0:
            xsh[:, 1:] = x[n, :, : HSH + 1]
            dsh[1:] = depth[n, 0, : HSH + 1]
        else:
            xsh[:, :-1] = x[n, :, HSH - 1 :]
            dsh[:-1] = depth[n, 0, HSH - 1 :]
        in_maps.append(
            {
                "xs": xsh,
                "dep": np.ascontiguousarray(dsh.reshape(-1)),
                "wt": wt9,
                "bb": bb,
            }
        )
    return in_maps


def gather_out(results):
    out = np.empty((N, O, H, W), np.float32)
    for cid in range(NCORES):
        n, hh = divmod(cid, 2)
        out[n, :, hh * HSH : (hh + 1) * HSH] = results[cid]["out"]
    return out


def kernel(x, depth, camera_params, weight, bias):
    nc = _get_nc()
    in_maps = make_in_maps(x, depth, weight, bias)
    res = run_bass_kernel_spmd(nc, in_maps, list(range(NCORES)))
    return gather_out(res.results)


# revision 39
# speedup vs baseline: 1.0034x; 1.0034x over previous
"""DepthAwareConv2d Trainium2 kernel.

Math: the reference's depth-modulated im2col GEMM is exactly
    out = conv2d(x * depth, weight, stride=1, pad=1) + bias
(depth broadcasts over channels; unfold(x)*unfold(depth) = unfold(x*depth)).

Sharding (8 cores): data-parallel over N (4 images) x spatial-parallel over
image row halves.  Core cid handles n = cid//2, row half = cid%2 (output rows
[0,64) or [64,128)), computing all 256 output channels for its half.  The
host ships each core its 64 input rows plus one halo/zero row on each side
(66 rows total), so the device program is identical on every core (SPMD) and
no collectives are needed.

Per-core device kernel:
  1. DMA the 66 x-rows and the matching partition-broadcast depth rows into
     SBUF chunks; DVE-multiply them into a column-padded fp32r image
     ypad (C=128 partitions, 66 x 130).
  2. Shift-conv: per 4-row output block and 128-wide out-channel block,
     9 accumulating fp32r matmuls (stationary = 128x128 weight tap, moving =
     shifted 4x128 window, free dim 512 = full-rate fp32 on the PE) into one
     PSUM bank.
  3. ScalarE Identity(+bias) PSUM->SBUF, DMA out.
"""

import numpy as np

import concourse.bass as bass
import concourse.mybir as mybir
import concourse.tile as tile
from concourse import bacc
from concourse.bass_utils import run_bass_kernel_spmd

N, C, O, H, W = 4, 128, 256, 128, 128
HSH = H // 2  # output rows per core
HIN = HSH + 2  # input rows per core incl. halo/zero rows
NCORES = 8
F32 = mybir.dt.float32
F32R = mybir.dt.float32r
ACT_IDENT = mybir.ActivationFunctionType.Identity

RB = 4  # output rows per matmul tile (free dim RB*W = 512, one PSUM bank)
# image rows per load/multiply chunk; a small first chunk lets the first
# matmul block (which only needs rows 0..5) start as early as possible
CHUNKS = (8, 8, 12, 12, 12, 14)  # sums to HIN = 66; boundaries all 0 mod 4

_CACHE = {}


def build_nc():
    nc = bacc.Bacc("TRN2", target_bir_lowering=False, debug=False, num_devices=NCORES)
    xs = nc.declare_dram_parameter("xs", [C, HIN, W], F32, isOutput=False)
    dep = nc.declare_dram_parameter("dep", [HIN * W], F32, isOutput=False)
    wt = nc.declare_dram_parameter("wt", [C, 2, 9, O // 2], F32, isOutput=False)
    bb = nc.declare_dram_parameter("bb", [O // 2, 2], F32, isOutput=False)
    out = nc.declare_dram_parameter("out", [O, HSH, W], F32, isOutput=True)

    with tile.TileContext(nc) as tc:
        with (
            tc.tile_pool(name="big", bufs=1) as big,
            tc.tile_pool(name="wp", bufs=1) as wp,
            tc.tile_pool(name="ch", bufs=3) as chp,
            tc.tile_pool(name="op", bufs=4) as op,
            tc.tile_pool(name="pp", bufs=8, space="PSUM") as pp,
        ):
            # fp32r tiles: every writer must emit fp32r-rounded values
            # (BIR verifier rule for FP32r matmult inputs), so x*depth and
            # the weights are written by compute ops with fp32r out dtype.
            ypad = big.tile([C, HIN, W + 2], F32R)
            wsb = wp.tile([C, 2, 9, O // 2], F32R)
            wtmp = wp.tile([C, 2, 9, O // 2], F32)
            bsb = wp.tile([O // 2, 2], F32)  # bsb[p, ob] = bias[ob*128 + p]

            # zeros staging: borders of ypad and the PE warm-up operand.
            # Memset can't write fp32r, so round-copy zeros via DVE.
            ztile = wp.tile([C, RB * W], F32)
            zr = wp.tile([C, RB * W], F32R)
            nc.vector.memset(ztile, 0.0)
            nc.vector.tensor_copy(out=zr, in_=ztile)
            nc.vector.tensor_copy(out=ypad[:, :, 0], in_=ztile[:, :HIN])
            nc.vector.tensor_copy(out=ypad[:, :, W + 1], in_=ztile[:, :HIN])

            # PE warm-up: ~4us of zero matmuls while the input DMA streams in,
            # so the HAM clock gate is already at full rate (2.4 GHz) when the
            # real matmuls start.
            # PE warm-up first: ~3.4us of zero fp32r matmuls brings the HAM
            # clock gate to full rate while the input DMA streams in.
            warm = pp.tile([O // 2, RB, W], F32, tag="ps")
            for _ in range(8):
                nc.tensor.matmul(warm, zr[:, :128], zr, start=True, stop=True)

            # depth rows of chunk 0 broadcast on the PE (a K=1 ones-matmul
            # replicates one depth row across all 128 partitions into PSUM),
            # so the startup chunk doesn't need its 0.5MB DMA broadcast in
            # the critical head window.  Runs after the warm-ups => full rate.
            NBC = 1  # chunks 0..NBC-1 use the PE broadcast
            bcrows = sum(CHUNKS[:NBC])
            dsm = wp.tile([1, bcrows * W], F32)
            ones1 = wp.tile([1, 128], F32)
            nc.sync.dma_start(out=dsm, in_=dep.ap()[None, : bcrows * W])
            nc.vector.memset(ones1, 1.0)
            dbc_ps = []
            for k in range(bcrows * W // 512):
                psb = pp.tile([C, RB, W], F32, tag="ps", name=f"psb{k}")
                nc.tensor.matmul(
                    psb, ones1, dsm[:, k * 512 : (k + 1) * 512], start=True, stop=True
                )
                dbc_ps.append(psb)

            CMAX = max(CHUNKS)
            bases = []
            b = 0
            for ch in CHUNKS:
                bases.append(b)
                b += ch
            tiles = {}

            def chunk_dma(ci):
                r0, ch = bases[ci], CHUNKS[ci]
                xb = chp.tile([C, CMAX, W], F32, tag="xb", name=f"xb{ci}")
                nc.sync.dma_start(out=xb[:, :ch], in_=xs[:, r0 : r0 + ch, :])
                if ci < NBC:
                    tiles[ci] = (xb, None)
                    return
                db = chp.tile([C, CMAX, W], F32, tag="db", name=f"db{ci}")
                nc.sync.dma_start(
                    out=db[:, :ch],
                    in_=dep.ap()[r0 * W : (r0 + ch) * W].partition_broadcast(C),
                )
                tiles[ci] = (xb, db)

            def mul_rows(r0, r1):
                # 4-aligned multiply blocks: conv block t reads ypad rows
                # 4t..4t+5; keeping writer granularity 4-aligned means Tile's
                # (quantized) range-overlap check never drags in a writer one
                # byte past the true read range.
                ci = next(
                    i for i, base in enumerate(bases) if base <= r0 < base + CHUNKS[i]
                )
                xb, db = tiles[ci]
                lo = r0 - bases[ci]
                if db is None:
                    d_ap = dbc_ps[r0 // 4][:, : r1 - r0]
                else:
                    d_ap = db[:, lo : lo + (r1 - r0)]
                nc.vector.tensor_mul(
                    out=ypad[:, r0:r1, 1 : W + 1],
                    in0=xb[:, lo : lo + (r1 - r0)],
                    in1=d_ap,
                )

            # Startup-critical transfers (weights half 0 + chunk 0) are split
            # into ~0.26MB pieces dual-dispatched on the two HWDGE engines
            # (sync + scalar): a single dma_start only sustains ~180GB/s and
            # each dispatch costs ~0.6us of sequencer time, so saturating the
            # ~435GB/s of SBUF ports needs several concurrent dma_starts.
            xb0 = chp.tile([C, CMAX, W], F32, tag="xb", name="xb0")
            tiles[0] = (xb0, None)
            ch0 = CHUNKS[0]
            nc.sync.dma_start(out=xb0[:, : ch0 // 2], in_=xs[:, : ch0 // 2, :])
            nc.scalar.dma_start(out=wtmp[:, 0, :5], in_=wt.ap()[:, 0, :5])
            nc.sync.dma_start(out=xb0[:, ch0 // 2 : ch0], in_=xs[:, ch0 // 2 : ch0, :])
            nc.scalar.copy(out=wsb[:, 0, :5], in_=wtmp[:, 0, :5])
            mul_rows(0, 4)
            mul_rows(4, 8)

            # second wave: weight tail, chunk 1, bias
            nc.scalar.dma_start(out=wtmp[:, 0, 5:], in_=wt.ap()[:, 0, 5:])
            nc.scalar.copy(out=wsb[:, 0, 5:], in_=wtmp[:, 0, 5:])
            nc.scalar.dma_start(out=wtmp[:, 1, :5], in_=wt.ap()[:, 1, :5])
            nc.scalar.dma_start(out=wtmp[:, 1, 5:], in_=wt.ap()[:, 1, 5:])
            chunk_dma(1)
            nc.scalar.copy(out=wsb[:, 1], in_=wtmp[:, 1])
            nc.scalar.dma_start(out=bsb, in_=bb.ap())

            for ci in range(2, len(CHUNKS)):
                with tc.tile_wait_until(0.004 * (ci - 1)):
                    chunk_dma(ci)
            r = CHUNKS[0]
            while r < HIN:
                r1 = min(r + 4, HIN)
                mul_rows(r, r1)
                r = r1

            for rb in range(0, HSH, RB):
                osb = op.tile([O // 2, 2, RB, W], F32)
                for ob in range(2):
                    ps = pp.tile([O // 2, RB, W], F32, tag="ps", name=f"ps{rb}_{ob}")
                    for p in range(9):
                        i, j = divmod(p, 3)
                        nc.tensor.matmul(
                            ps,
                            wsb[:, ob, p],
                            ypad[:, rb + i : rb + i + RB, j : j + W],
                            start=(p == 0),
                            stop=(p == 8),
                        )
                    nc.scalar.activation(
                        out=osb[:, ob],
                        in_=ps,
                        func=ACT_IDENT,
                        bias=bsb[:, ob : ob + 1],
                        scale=1.0,
                    )
                # osb holds (128 partitions, [ob, row, col]); DRAM wants
                # (o, row, col) with o = ob*128 + partition.
                nc.sync.dma_start(
                    out=out[:, rb : rb + RB, :].rearrange(
                        "(ob o) r w -> o ob r w", ob=2
                    ),
                    in_=osb,
                )

    nc.compile()
    return nc


def _get_nc():
    if "nc" not in _CACHE:
        _CACHE["nc"] = build_nc()
    return _CACHE["nc"]


def make_in_maps(x, depth, weight, bias):
    x = np.asarray(x, np.float32)
    depth = np.asarray(depth, np.float32)
    weight = np.asarray(weight, np.float32)
    bias = np.asarray(bias, np.float32)
    # (O, C, 3, 3) -> (C, ob, tap=i*3+j, o) with o = local index in the
    # 128-wide out-channel half ob
    wt9 = np.ascontiguousarray(
        np.transpose(
            weight.reshape(2, O // 2, C, 3, 3), (2, 0, 3, 4, 1)
        ).reshape(C, 2, 9, O // 2)
    )
    bb = np.ascontiguousarray(bias.reshape(2, O // 2).T)
    in_maps = []
    for cid in range(NCORES):
        n, hh = divmod(cid, 2)
        xsh = np.zeros((C, HIN, W), np.float32)
        dsh = np.zeros((HIN, W), np.float32)
        if hh == 0:
            xsh[:, 1:] = x[n, :, : HSH + 1]
            dsh[1:] = depth[n, 0, : HSH + 1]
        else:
            xsh[:, :-1] = x[n, :, HSH - 1 :]
            dsh[:-1] = depth[n, 0, HSH - 1 :]
        in_maps.append(
            {
                "xs": xsh,
                "dep": np.ascontiguousarray(dsh.reshape(-1)),
                "wt": wt9,
                "bb": bb,
            }
        )
    return in_maps


def gather_out(results):
    out = np.empty((N, O, H, W), np.float32)
    for cid in range(NCORES):
        n, hh = divmod(cid, 2)
        out[n, :, hh * HSH : (hh + 1) * HSH] = results[cid]["out"]
    return out


def kernel(x, depth, camera_params, weight, bias):
    nc = _get_nc()
    in_maps = make_in_maps(x, depth, weight, bias)
    res = run_bass_kernel_spmd(nc, in_maps, list(range(NCORES)))
    return gather_out(res.results)
